# revision 42
# baseline (speedup 1.0000x reference)
"""Cost-volume kernel for Trainium2 (Bass/Tile), 8-core SPMD.

Problem: left/right features [B=2, C=32, H=128, W=256] f32.
Output [B, 2C=64, D=48, H, W] where for disparity d in [-8, 40):
  out[:, 0:C,  d+8, h, x] = left[:, :, h, x]   if 0 <= x-d < W else 0
  out[:, C:2C, d+8, h, x] = right[:, :, h, x-d] if 0 <= x-d < W else 0

Pure data movement, bound by HBM store bandwidth. Design (measured
values from NTFF traces on this instance):

  - fp16 end-to-end: host quantizes inputs to fp16, device moves fp16,
    host upcasts to f32. Quantization rel-err ~3.6e-4, far inside the
    2e-2 gate. Halves HBM bytes vs f32.
  - H-row sharding: 16 rows of H per core; each core builds the full
    disparity volume for all 64 channels of its row band.
  - Packed output: slice d only has W-|d| valid columns; the device
    writes just those, back-to-back per partition (descriptors stay
    3.4-4 KiB). The host drops each slab into a np.zeros output, so
    the zero triangles are never moved over HBM (-6.6% bytes).
  - Stores go via the two HWDGE rings (left slices on nc.scalar,
    right slices on nc.sync; 8 SDMA engines each, byte-balanced by
    construction). HWDGE descriptor generation is RTL, immune to the
    DVE 2-port perf-mode lock that starves SWDGE (gpsimd Q7) emission
    whenever DVE tensor_copy runs. Sustained 406-414 GB/s combined.
  - The d=0 slices equal the inputs verbatim, so the host places them
    directly and the device never moves those bytes (-2 MB writes per
    core). DRAM->DRAM ring-head fillers for them were tried first:
    once every engine is 100% busy end-to-end (which the packed
    layout achieves), the filler's extra 1 MB input re-read costs
    more engine time than the ramp idle it hides.
  - Loads stay on gpsimd SWDGE: spread over all 16 engines, so they
    do not skew the two HWDGE rings.
  - Every staged slice is a DVE tensor_copy of the valid window into
    a compact staging tile, emitted in store order (L,R,L,R by
    growing |d|), so the DVE feeds both rings evenly at ~2x the
    per-ring store cadence.
"""

import numpy as np

B, C, H, W = 2, 32, 128, 256
MIN_D, MAX_D = -8, 40
D = MAX_D - MIN_D  # 48
N_CORES = 8
HB = H // N_CORES  # 16 rows of H per core

HL = 8             # h rows held per partition
HH = HB // HL      # 2
NPART = B * C * HH  # 128 partitions: p = (b*C + c)*HH + h_hi

STAGE_BUFS = 8  # staging rotation depth in 4-slice tiles (32 slices)

# packed offsets: slice di occupies HL*(W - |d|) elements per partition.
# d=0 (di=8) takes no slot: its slices equal the inputs verbatim and the
# host places them straight from the (already-quantized) input arrays,
# so the device never moves those bytes at all.
OFF = [0]
for _di in range(D):
    _w = 0 if _di == -MIN_D else W - abs(_di + MIN_D)
    OFF.append(OFF[-1] + HL * _w)
PACK = OFF[-1]  # 89728 elements per partition

# Slices are stored in MERGED GROUPS OF FOUR: adjacent di are contiguous
# in the packed layout (di=8 is zero-width), so four slices share one
# ~2 MB dma_start with 14-16 KiB per-partition runs - 4x fewer
# descriptors and completion semaphores than per-slice stores (the
# pair-merge step alone measured -4.9 us). Groups emit widest-first.
_dis = [di for di in range(D) if di != -MIN_D]
GROUPS = [_dis[i : i + 4] for i in range(0, len(_dis), 4)]
GROUPS.sort(key=lambda g: min(abs(di + MIN_D) for di in g))

_CACHE = {}


def _build_nc():
    import concourse.bacc as bacc
    import concourse.tile as tile
    import concourse.mybir as mybir

    f16 = mybir.dt.float16

    nc = bacc.Bacc(
        "TRN2",
        target_bir_lowering=False,
        debug=False,
        enable_asserts=False,
        num_devices=N_CORES,
    )
    left_in = nc.dram_tensor("left_in", [B, C, HB, W], f16, kind="ExternalInput")
    right_in = nc.dram_tensor("right_in", [B, C, HB, W], f16, kind="ExternalInput")
    left_pack = nc.dram_tensor("left_pack", [NPART, PACK], f16, kind="ExternalOutput")
    right_pack = nc.dram_tensor("right_pack", [NPART, PACK], f16, kind="ExternalOutput")

    with tile.TileContext(nc) as tc:
        with (
            tc.tile_pool(name="pool", bufs=1) as pool,
            tc.tile_pool(name="stpool", bufs=STAGE_BUFS) as stpool,
        ):
            lt = pool.tile([NPART, HL * W], f16, tag="lt")
            rt = pool.tile([NPART, HL * W], f16, tag="rt")
            l3 = lt[:].rearrange("p (h w) -> p h w", h=HL)
            r3 = rt[:].rearrange("p (h w) -> p h w", h=HL)
            # loads at the HWDGE ring heads: the rings wake ~1 us before
            # the gpsimd Q7 finishes its preamble, and each ring's stores
            # queue behind its own load anyway. Left on Sync (wakes
            # earliest; left copies are first in the DVE order), right on
            # Scalar. Keeps the rings byte-balanced (+0.5 MB each).
            nc.sync.dma_start(lt[:], left_in.ap())
            nc.scalar.dma_start(rt[:], right_in.ap())

            def emit_group(g, src3, out_t, engine, is_left):
                # valid output columns are x in [x0, x0+w); the source
                # window within the (unpadded) input row:
                #   left slice d:  value left[x]   -> cols [max(0,d), ...)
                #   right slice d: value right[x-d]-> cols [max(0,-d), ...)
                stage = stpool.tile([NPART, 4 * HL * W], f16, tag="st")
                o = 0
                for di in g:
                    d = di + MIN_D
                    w = W - abs(d)
                    s0 = max(0, d) if is_left else max(0, -d)
                    st3 = stage[:, o : o + HL * w].rearrange(
                        "p (h w) -> p h w", h=HL
                    )
                    nc.vector.tensor_copy(st3[:], src3[:, :, s0 : s0 + w])
                    o += HL * w
                engine.dma_start(
                    out_t.ap()[:, OFF[g[0]] : OFF[g[0]] + o], stage[:, 0:o]
                )

            for g in GROUPS:
                emit_group(g, l3, left_pack, nc.scalar, True)
                emit_group(g, r3, right_pack, nc.sync, False)

    nc.compile()
    return nc


def _get_nc():
    if "nc" not in _CACHE:
        _CACHE["nc"] = _build_nc()
    return _CACHE["nc"]


def kernel(left_feat, right_feat):
    from concourse.bass_utils import run_bass_kernel_spmd

    left = np.asarray(left_feat)
    right = np.asarray(right_feat)
    assert left.shape == (B, C, H, W) and right.shape == (B, C, H, W)

    nc = _get_nc()
    left16 = left.astype(np.float16)
    right16 = right.astype(np.float16)
    in_maps = []
    for m in range(N_CORES):
        rows = slice(m * HB, (m + 1) * HB)
        in_maps.append(
            {
                "left_in": np.ascontiguousarray(left16[:, :, rows, :]),
                "right_in": np.ascontiguousarray(right16[:, :, rows, :]),
            }
        )
    res = run_bass_kernel_spmd(nc, in_maps, core_ids=list(range(N_CORES))).results

    # np.zeros is calloc-backed: the zero triangles the device never
    # writes stay as untouched zero pages.
    out = np.zeros((B, 2 * C, D, H, W), dtype=np.float32)
    # d=0 slices are the inputs verbatim - placed from the original f32
    # arrays (exact), never moved over device HBM.
    out[:, :C, -MIN_D] = left
    out[:, C:, -MIN_D] = right
    for m in range(N_CORES):
        rows = slice(m * HB, (m + 1) * HB)
        lp = res[m]["left_pack"].reshape(B, C, HH, PACK)
        rp = res[m]["right_pack"].reshape(B, C, HH, PACK)
        for di in range(D):
            d = di + MIN_D
            if d == 0:
                continue
            w = W - abs(d)
            x0 = max(0, d)
            seg = lp[:, :, :, OFF[di] : OFF[di] + HL * w].reshape(B, C, HB, w)
            out[:, :C, di, rows, x0 : x0 + w] = seg
            seg = rp[:, :, :, OFF[di] : OFF[di] + HL * w].reshape(B, C, HB, w)
            out[:, C:, di, rows, x0 : x0 + w] = seg
    return out


# revision 45
# speedup vs baseline: 1.0025x; 1.0025x over previous
"""Cost-volume kernel for Trainium2 (Bass/Tile), 8-core SPMD.

Problem: left/right features [B=2, C=32, H=128, W=256] f32.
Output [B, 2C=64, D=48, H, W] where for disparity d in [-8, 40):
  out[:, 0:C,  d+8, h, x] = left[:, :, h, x]   if 0 <= x-d < W else 0
  out[:, C:2C, d+8, h, x] = right[:, :, h, x-d] if 0 <= x-d < W else 0

Pure data movement, bound by HBM store bandwidth. Design (measured
values from NTFF traces on this instance):

  - fp16 end-to-end: host quantizes inputs to fp16, device moves fp16,
    host upcasts to f32. Quantization rel-err ~3.6e-4, far inside the
    2e-2 gate. Halves HBM bytes vs f32.
  - H-row sharding: 16 rows of H per core; each core builds the full
    disparity volume for all 64 channels of its row band.
  - Packed output: slice d only has W-|d| valid columns; the device
    writes just those, back-to-back per partition (descriptors stay
    3.4-4 KiB). The host drops each slab into a np.zeros output, so
    the zero triangles are never moved over HBM (-6.6% bytes).
  - Stores go via the two HWDGE rings (left slices on nc.scalar,
    right slices on nc.sync; 8 SDMA engines each, byte-balanced by
    construction). HWDGE descriptor generation is RTL, immune to the
    DVE 2-port perf-mode lock that starves SWDGE (gpsimd Q7) emission
    whenever DVE tensor_copy runs. Sustained 406-414 GB/s combined.
  - The d=0 slices equal the inputs verbatim, so the host places them
    directly and the device never moves those bytes (-2 MB writes per
    core). DRAM->DRAM ring-head fillers for them were tried first:
    once every engine is 100% busy end-to-end (which the packed
    layout achieves), the filler's extra 1 MB input re-read costs
    more engine time than the ramp idle it hides.
  - Loads stay on gpsimd SWDGE: spread over all 16 engines, so they
    do not skew the two HWDGE rings.
  - Every staged slice is a DVE tensor_copy of the valid window into
    a compact staging tile, emitted in store order (L,R,L,R by
    growing |d|), so the DVE feeds both rings evenly at ~2x the
    per-ring store cadence.
"""

import numpy as np

B, C, H, W = 2, 32, 128, 256
MIN_D, MAX_D = -8, 40
D = MAX_D - MIN_D  # 48
N_CORES = 8
HB = H // N_CORES  # 16 rows of H per core

HL = 8             # h rows held per partition
HH = HB // HL      # 2
NPART = B * C * HH  # 128 partitions: p = (b*C + c)*HH + h_hi

STAGE_BUFS = 16  # staging rotation depth in slice-pair tiles (32 slices)

# packed offsets: slice di occupies HL*(W - |d|) elements per partition.
# d=0 (di=8) takes no slot: its slices equal the inputs verbatim and the
# host places them straight from the (already-quantized) input arrays,
# so the device never moves those bytes at all.
OFF = [0]
for _di in range(D):
    _w = 0 if _di == -MIN_D else W - abs(_di + MIN_D)
    OFF.append(OFF[-1] + HL * _w)
PACK = OFF[-1]  # 89728 elements per partition

# Slices are stored in MERGED PAIRS: adjacent di are contiguous in the
# packed layout (di=8 is zero-width), so two slices share one ~1 MB
# dma_start with 7-8 KiB per-partition runs - half the descriptors and
# completion semaphores of per-slice stores (measured -4.9 us; merging
# four per store measured no further gain). Groups emit widest-first.
_dis = [di for di in range(D) if di != -MIN_D]
GROUPS = [_dis[i : i + 2] for i in range(0, len(_dis), 2)]
GROUPS.sort(key=lambda g: min(abs(di + MIN_D) for di in g))

_CACHE = {}


def _build_nc():
    import concourse.bacc as bacc
    import concourse.tile as tile
    import concourse.mybir as mybir

    f16 = mybir.dt.float16

    nc = bacc.Bacc(
        "TRN2",
        target_bir_lowering=False,
        debug=False,
        enable_asserts=False,
        num_devices=N_CORES,
    )
    left_in = nc.dram_tensor("left_in", [B, C, HB, W], f16, kind="ExternalInput")
    right_in = nc.dram_tensor("right_in", [B, C, HB, W], f16, kind="ExternalInput")
    left_pack = nc.dram_tensor("left_pack", [NPART, PACK], f16, kind="ExternalOutput")
    right_pack = nc.dram_tensor("right_pack", [NPART, PACK], f16, kind="ExternalOutput")

    with tile.TileContext(nc) as tc:
        with (
            tc.tile_pool(name="pool", bufs=1) as pool,
            tc.tile_pool(name="stpool", bufs=STAGE_BUFS) as stpool,
        ):
            lt = pool.tile([NPART, HL * W], f16, tag="lt")
            rt = pool.tile([NPART, HL * W], f16, tag="rt")
            l3 = lt[:].rearrange("p (h w) -> p h w", h=HL)
            r3 = rt[:].rearrange("p (h w) -> p h w", h=HL)
            # loads at the HWDGE ring heads: the rings wake ~1 us before
            # the gpsimd Q7 finishes its preamble, and each ring's stores
            # queue behind its own load anyway. Left on Sync (wakes
            # earliest; left copies are first in the DVE order), right on
            # Scalar. Keeps the rings byte-balanced (+0.5 MB each).
            nc.sync.dma_start(lt[:], left_in.ap())
            nc.scalar.dma_start(rt[:], right_in.ap())

            def emit_group(g, src3, out_t, engine, is_left):
                # valid output columns are x in [x0, x0+w); the source
                # window within the (unpadded) input row:
                #   left slice d:  value left[x]   -> cols [max(0,d), ...)
                #   right slice d: value right[x-d]-> cols [max(0,-d), ...)
                stage = stpool.tile([NPART, 2 * HL * W], f16, tag="st")
                o = 0
                for di in g:
                    d = di + MIN_D
                    w = W - abs(d)
                    s0 = max(0, d) if is_left else max(0, -d)
                    st3 = stage[:, o : o + HL * w].rearrange(
                        "p (h w) -> p h w", h=HL
                    )
                    nc.vector.tensor_copy(st3[:], src3[:, :, s0 : s0 + w])
                    o += HL * w
                engine.dma_start(
                    out_t.ap()[:, OFF[g[0]] : OFF[g[0]] + o], stage[:, 0:o]
                )

            for g in GROUPS:
                emit_group(g, l3, left_pack, nc.scalar, True)
                emit_group(g, r3, right_pack, nc.sync, False)

    nc.compile()
    return nc


def _get_nc():
    if "nc" not in _CACHE:
        _CACHE["nc"] = _build_nc()
    return _CACHE["nc"]


def kernel(left_feat, right_feat):
    from concourse.bass_utils import run_bass_kernel_spmd

    left = np.asarray(left_feat)
    right = np.asarray(right_feat)
    assert left.shape == (B, C, H, W) and right.shape == (B, C, H, W)

    nc = _get_nc()
    left16 = left.astype(np.float16)
    right16 = right.astype(np.float16)
    in_maps = []
    for m in range(N_CORES):
        rows = slice(m * HB, (m + 1) * HB)
        in_maps.append(
            {
                "left_in": np.ascontiguousarray(left16[:, :, rows, :]),
                "right_in": np.ascontiguousarray(right16[:, :, rows, :]),
            }
        )
    res = run_bass_kernel_spmd(nc, in_maps, core_ids=list(range(N_CORES))).results

    # np.zeros is calloc-backed: the zero triangles the device never
    # writes stay as untouched zero pages.
    out = np.zeros((B, 2 * C, D, H, W), dtype=np.float32)
    # d=0 slices are the inputs verbatim - placed from the original f32
    # arrays (exact), never moved over device HBM.
    out[:, :C, -MIN_D] = left
    out[:, C:, -MIN_D] = right
    for m in range(N_CORES):
        rows = slice(m * HB, (m + 1) * HB)
        lp = res[m]["left_pack"].reshape(B, C, HH, PACK)
        rp = res[m]["right_pack"].reshape(B, C, HH, PACK)
        for di in range(D):
            d = di + MIN_D
            if d == 0:
                continue
            w = W - abs(d)
            x0 = max(0, d)
            seg = lp[:, :, :, OFF[di] : OFF[di] + HL * w].reshape(B, C, HB, w)
            out[:, :C, di, rows, x0 : x0 + w] = seg
            seg = rp[:, :, :, OFF[di] : OFF[di] + HL * w].reshape(B, C, HB, w)
            out[:, C:, di, rows, x0 : x0 + w] = seg
    return out
